# revision 12
# baseline (speedup 1.0000x reference)
"""Trainium2 Bass kernel for a dense transformer block.

B=64, T=256, C=1024, H=16, HD=64. Data-parallel over batch across 8 cores
(8 batch elements per core). All heavy matmuls run in fp32r (full PE rate);
attention is computed in scores-transposed layout so the softmax reduction
is a PE ones-matmul and no attention-matrix transpose is needed.
"""

import sys

if "/opt/trn_rl_repo" not in sys.path:
    sys.path.insert(0, "/opt/trn_rl_repo")

import numpy as np

import concourse.bass as bass
import concourse.tile as tile
from concourse import bacc, mybir
from concourse.bass_utils import run_bass_kernel_spmd

f32 = mybir.dt.float32
f32r = mybir.dt.float32r
AF = mybir.ActivationFunctionType
AL = mybir.AluOpType

NCORES = 8
BB = 8          # batch elements per core
T = 256
C = 1024
H = 16
HD = 64
FF = 4096
EPS = 1e-5

_cache = {}


def _layernorm_transpose(nc, x_tile, g_sb, b_sb, hT_tile, ident_sb, small, ps_tr,
                         eps_sb):
    """x_tile [128,2,1024] f32 natural (t on partitions, modified in place) ->
    hT_tile [128,8,256] f32r transposed (c on partitions) with affine applied."""
    for tcb in range(2):
        stats = small.tile([128, 2, 6], f32, tag="stats")
        nc.vector.bn_stats(stats[:, 0, :], x_tile[:, tcb, 0:512])
        nc.vector.bn_stats(stats[:, 1, :], x_tile[:, tcb, 512:1024])
        mv = small.tile([128, 2], f32, tag="mv")
        nc.vector.bn_aggr(mv[:, :], stats[:, :, :])
        sd = small.tile([128, 1], f32, tag="sd")
        nc.scalar.activation(sd[:, :], mv[:, 1:2], AF.Sqrt, bias=eps_sb[:, :])
        rstd = small.tile([128, 1], f32, tag="rstd")
        nc.vector.reciprocal(rstd[:, :], sd[:, :])
        nc.vector.tensor_scalar(
            out=x_tile[:, tcb, :], in0=x_tile[:, tcb, :],
            scalar1=mv[:, 0:1], scalar2=rstd[:, :],
            op0=AL.subtract, op1=AL.mult,
        )
    for tcb in range(2):
        for cc in range(8):
            pt = ps_tr.tile([128, 128], f32, tag="tr")
            nc.tensor.transpose(pt[:, :], x_tile[:, tcb, cc * 128:(cc + 1) * 128],
                                ident_sb[:, :])
            nc.scalar.activation(
                out=hT_tile[:, cc, tcb * 128:(tcb + 1) * 128], in_=pt[:, :],
                func=AF.Identity, scale=g_sb[:, cc:cc + 1], bias=b_sb[:, cc:cc + 1],
            )


def _build():
    nc = bacc.Bacc("TRN2", target_bir_lowering=False, debug=False,
                   num_devices=NCORES)

    # ---- external IO (host pre-rearranged layouts) ----
    x_d = nc.dram_tensor("x_r", [BB, 128, 2, C], f32, kind="ExternalInput")
    wq_d = nc.dram_tensor("wq_r", [128, 8, C], f32, kind="ExternalInput")
    wk_d = nc.dram_tensor("wk_r", [128, 8, C], f32, kind="ExternalInput")
    wv_d = nc.dram_tensor("wv_r", [128, 8, C], f32, kind="ExternalInput")
    wp_d = nc.dram_tensor("wp_r", [128, 8, C], f32, kind="ExternalInput")
    w1_d = nc.dram_tensor("w1_r", [128, 8, FF], f32, kind="ExternalInput")
    w2_d = nc.dram_tensor("w2_r", [128, 32, C], f32, kind="ExternalInput")
    ln1g_d = nc.dram_tensor("ln1g_r", [128, 8], f32, kind="ExternalInput")
    ln1b_d = nc.dram_tensor("ln1b_r", [128, 8], f32, kind="ExternalInput")
    ln2g_d = nc.dram_tensor("ln2g_r", [128, 8], f32, kind="ExternalInput")
    ln2b_d = nc.dram_tensor("ln2b_r", [128, 8], f32, kind="ExternalInput")
    b1_d = nc.dram_tensor("b1_r", [128, 32], f32, kind="ExternalInput")
    bproj_d = nc.dram_tensor("bproj_bc", [128, C], f32, kind="ExternalInput")
    b2_d = nc.dram_tensor("b2_bc", [128, C], f32, kind="ExternalInput")
    cmask_d = nc.dram_tensor("cmask", [128, 256], f32, kind="ExternalInput")
    cmask1_d = nc.dram_tensor("cmask1", [128, 256], f32, kind="ExternalInput")
    ident_d = nc.dram_tensor("ident", [128, 128], f32, kind="ExternalInput")
    ones128_d = nc.dram_tensor("ones128", [128, 1], f32, kind="ExternalInput")
    ones64_d = nc.dram_tensor("ones64", [1, 64], f32, kind="ExternalInput")
    out_d = nc.dram_tensor("out_r", [BB, 128, 2, C], f32, kind="ExternalOutput")

    # ---- internal DRAM scratch (per-phase spills) ----
    oT_d = nc.dram_tensor("oT_s", [BB, 128, 8, T], f32r, kind="Internal")
    x2_d = nc.dram_tensor("x2_s", [BB, 128, 2, C], f32, kind="Internal")
    h2T_d = nc.dram_tensor("h2T_s", [BB, 128, 8, T], f32r, kind="Internal")
    y1g_d = nc.dram_tensor("y1g_s", [BB, 128, 32, T], f32r, kind="Internal")

    with tile.TileContext(nc) as tc:
        with tc.tile_pool(name="const", bufs=1) as cpool:
            ident_sb = cpool.tile([128, 128], f32, tag="ident")
            nc.sync.dma_start(ident_sb[:, :], ident_d[:, :])
            cmask_sb = cpool.tile([128, 256], f32r, tag="cmask")
            nc.sync.dma_start(cmask_sb[:, :], cmask_d[:, :].bitcast(f32r))
            cmask1_sb = cpool.tile([128, 256], f32r, tag="cmask1")
            nc.sync.dma_start(cmask1_sb[:, :], cmask1_d[:, :].bitcast(f32r))
            ones128_sb = cpool.tile([128, 1], f32r, tag="ones128")
            nc.sync.dma_start(ones128_sb[:, :], ones128_d[:, :].bitcast(f32r))
            ones64_sb = cpool.tile([1, 64], f32r, tag="ones64")
            nc.sync.dma_start(ones64_sb[:, :], ones64_d[:, :].bitcast(f32r))
            ln1g_sb = cpool.tile([128, 8], f32, tag="ln1g")
            nc.sync.dma_start(ln1g_sb[:, :], ln1g_d[:, :])
            ln1b_sb = cpool.tile([128, 8], f32, tag="ln1b")
            nc.sync.dma_start(ln1b_sb[:, :], ln1b_d[:, :])
            ln2g_sb = cpool.tile([128, 8], f32, tag="ln2g")
            nc.sync.dma_start(ln2g_sb[:, :], ln2g_d[:, :])
            ln2b_sb = cpool.tile([128, 8], f32, tag="ln2b")
            nc.sync.dma_start(ln2b_sb[:, :], ln2b_d[:, :])
            b1_sb = cpool.tile([128, 32], f32, tag="b1")
            nc.sync.dma_start(b1_sb[:, :], b1_d[:, :])
            bproj_sb = cpool.tile([128, C], f32, tag="bproj")
            nc.sync.dma_start(bproj_sb[:, :], bproj_d[:, :])
            b2_sb = cpool.tile([128, C], f32, tag="b2")
            nc.sync.dma_start(b2_sb[:, :], b2_d[:, :])
            eps_sb = cpool.tile([128, 1], f32, tag="eps")
            nc.vector.memset(eps_sb[:, :], EPS)

            # ================= Phase A1: LN1 + QKV + attention =================
            with tc.tile_pool(name="wqkv", bufs=1) as wpool, \
                 tc.tile_pool(name="a1x", bufs=2) as a1x, \
                 tc.tile_pool(name="a1h", bufs=2) as a1h, \
                 tc.tile_pool(name="a1qkv", bufs=1) as a1qkv, \
                 tc.tile_pool(name="a1e", bufs=4) as a1e, \
                 tc.tile_pool(name="a1r", bufs=4) as a1r, \
                 tc.tile_pool(name="a1rb", bufs=2) as a1rb, \
                 tc.tile_pool(name="a1o", bufs=2) as a1o, \
                 tc.tile_pool(name="a1small", bufs=4) as a1small, \
                 tc.tile_pool(name="psa_tr", bufs=1, space="PSUM") as ps_tr, \
                 tc.tile_pool(name="psa_mm", bufs=2, space="PSUM") as ps_mm, \
                 tc.tile_pool(name="psa_po", bufs=2, space="PSUM") as ps_po, \
                 tc.tile_pool(name="psa_v", bufs=1, space="PSUM") as ps_v, \
                 tc.tile_pool(name="psa_pb", bufs=1, space="PSUM") as ps_pb, \
                 tc.tile_pool(name="psa_d", bufs=1, space="PSUM") as ps_d:

                wq_sb = wpool.tile([128, 8, C], f32r, tag="wq")
                wk_sb = wpool.tile([128, 8, C], f32r, tag="wk")
                wv_sb = wpool.tile([128, 8, C], f32r, tag="wv")
                for cc in range(8):
                    nc.sync.dma_start(wq_sb[:, cc, :], wq_d[:, cc, :].bitcast(f32r))
                    nc.sync.dma_start(wk_sb[:, cc, :], wk_d[:, cc, :].bitcast(f32r))
                    nc.sync.dma_start(wv_sb[:, cc, :], wv_d[:, cc, :].bitcast(f32r))

                for b in range(BB):
                    xn = a1x.tile([128, 2, C], f32, tag="xn")
                    nc.sync.dma_start(xn[:, :, :], x_d[b, :, :, :])
                    hT = a1h.tile([128, 8, T], f32r, tag="hT")
                    _layernorm_transpose(nc, xn, ln1g_sb, ln1b_sb, hT,
                                         ident_sb, a1small, ps_tr, eps_sb)

                    qT = a1qkv.tile([128, 8, T], f32r, tag="qT")
                    kT = a1qkv.tile([128, 8, T], f32r, tag="kT")
                    v_sb = a1qkv.tile([128, 2, C], f32r, tag="v")
                    for hp in range(8):
                        pq = ps_mm.tile([128, 256], f32, tag="mm")
                        for cc in range(8):
                            nc.tensor.matmul(pq[:, :],
                                             wq_sb[:, cc, hp * 128:(hp + 1) * 128],
                                             hT[:, cc, :],
                                             start=cc == 0, stop=cc == 7)
                        nc.vector.tensor_copy(qT[:, hp, :], pq[:, :])
                        pk = ps_mm.tile([128, 256], f32, tag="mm")
                        for cc in range(8):
                            nc.tensor.matmul(pk[:, :],
                                             wk_sb[:, cc, hp * 128:(hp + 1) * 128],
                                             hT[:, cc, :],
                                             start=cc == 0, stop=cc == 7)
                        nc.vector.tensor_copy(kT[:, hp, :], pk[:, :])
                    for sc in range(2):
                        for vh in range(2):
                            pv = ps_v.tile([128, 512], f32, tag="v")
                            for cc in range(8):
                                nc.tensor.matmul(pv[:, :],
                                                 hT[:, cc, sc * 128:(sc + 1) * 128],
                                                 wv_sb[:, cc, vh * 512:(vh + 1) * 512],
                                                 start=cc == 0, stop=cc == 7)
                            nc.vector.tensor_copy(v_sb[:, sc, vh * 512:(vh + 1) * 512],
                                                  pv[:, :])

                    oT = a1o.tile([128, 8, T], f32r, tag="oT")
                    for hp in range(8):
                        for hl in range(2):
                            h = 2 * hp + hl
                            dsl = slice(hl * 64, (hl + 1) * 64)
                            eh = a1e.tile([128, 2, 256], f32r, tag="exp")
                            for sc in range(2):
                                ps_s = ps_mm.tile([128, 256], f32, tag="mm")
                                nc.tensor.matmul(
                                    ps_s[:, :],
                                    kT[dsl, hp, sc * 128:(sc + 1) * 128],
                                    qT[dsl, hp, :],
                                    start=True, stop=True)
                                nc.scalar.activation(out=eh[:, sc, :], in_=ps_s[:, :],
                                                     func=AF.Exp, scale=0.125)
                            nc.vector.tensor_mul(eh[:, 0, :], eh[:, 0, :],
                                                 cmask_sb[:, :])
                            nc.vector.tensor_mul(eh[:, 1, :], eh[:, 1, :],
                                                 cmask1_sb[:, :])
                            pd = ps_d.tile([1, 256], f32, tag="d")
                            nc.tensor.matmul(pd[:, :], ones128_sb[:, :], eh[:, 0, :],
                                             start=True, stop=False)
                            nc.tensor.matmul(pd[:, :], ones128_sb[:, :], eh[:, 1, :],
                                             start=False, stop=True)
                            rc = a1r.tile([1, 256], f32r, tag="rcp")
                            with nc.allow_low_precision(
                                    reason="f32r is fp32-width; rounding only"
                                    " at the PE broadcast matmul"):
                                nc.vector.reciprocal(rc[:, :], pd[:, :])
                            # o_raw^T for this head: [64, 256] psum
                            poh = ps_po.tile([64, 256], f32, tag="po")
                            for sc in range(2):
                                nc.tensor.matmul(
                                    poh[:, :],
                                    v_sb[:, sc, h * 64:(h + 1) * 64],
                                    eh[:, sc, :],
                                    start=sc == 0, stop=sc == 1)
                            # broadcast 1/denom to 64 partitions via PE
                            pbh = ps_pb.tile([64, 256], f32, tag="pb")
                            nc.tensor.matmul(pbh[:, :], ones64_sb[:, :], rc[:, :],
                                             start=True, stop=True)
                            rbh = a1rb.tile([64, 256], f32, tag="rb")
                            nc.vector.tensor_copy(rbh[:, :], pbh[:, :])
                            if hl == 0:
                                nc.vector.tensor_mul(oT[0:64, hp, :],
                                                     poh[:, :], rbh[:, :])
                            else:
                                tmpb = a1rb.tile([64, 256], f32r, tag="tmpb")
                                nc.vector.tensor_mul(tmpb[:, :], poh[:, :], rbh[:, :])
                                nc.sync.dma_start(oT[64:128, hp, :], tmpb[:, :])
                    nc.sync.dma_start(oT_d[b, :, :, :], oT[:, :, :])

            # ================= Phase A2: proj + residual + LN2 =================
            with tc.tile_pool(name="wp", bufs=1) as wppool, \
                 tc.tile_pool(name="a2o", bufs=2) as a2o, \
                 tc.tile_pool(name="a2x", bufs=2) as a2x, \
                 tc.tile_pool(name="a2x2", bufs=2) as a2x2, \
                 tc.tile_pool(name="a2h", bufs=2) as a2h, \
                 tc.tile_pool(name="a2small", bufs=4) as a2small, \
                 tc.tile_pool(name="psa2_tr", bufs=2, space="PSUM") as ps2_tr, \
                 tc.tile_pool(name="psa2", bufs=4, space="PSUM") as ps_a2:

                wp_sb = wppool.tile([128, 8, C], f32r, tag="wp")
                for cc in range(8):
                    nc.sync.dma_start(wp_sb[:, cc, :], wp_d[:, cc, :].bitcast(f32r))

                for b in range(BB):
                    oT_l = a2o.tile([128, 8, T], f32r, tag="oTl")
                    nc.sync.dma_start(oT_l[:, :, :], oT_d[b, :, :, :])
                    xn = a2x.tile([128, 2, C], f32, tag="xn2")
                    nc.sync.dma_start(xn[:, :, :], x_d[b, :, :, :])
                    for tcb in range(2):
                        nc.vector.tensor_add(xn[:, tcb, :], xn[:, tcb, :],
                                             bproj_sb[:, :])
                    x2 = a2x2.tile([128, 2, C], f32, tag="x2")
                    for tcb in range(2):
                        for ch in range(2):
                            pp = ps_a2.tile([128, 512], f32, tag="pj")
                            for cc in range(8):
                                nc.tensor.matmul(
                                    pp[:, :],
                                    oT_l[:, cc, tcb * 128:(tcb + 1) * 128],
                                    wp_sb[:, cc, ch * 512:(ch + 1) * 512],
                                    start=cc == 0, stop=cc == 7)
                            nc.vector.tensor_add(x2[:, tcb, ch * 512:(ch + 1) * 512],
                                                 pp[:, :],
                                                 xn[:, tcb, ch * 512:(ch + 1) * 512])
                    nc.sync.dma_start(x2_d[b, :, :, :], x2[:, :, :])
                    h2T = a2h.tile([128, 8, T], f32r, tag="h2T")
                    _layernorm_transpose(nc, x2, ln2g_sb, ln2b_sb, h2T,
                                         ident_sb, a2small, ps2_tr, eps_sb)
                    nc.sync.dma_start(h2T_d[b, :, :, :], h2T[:, :, :])

            # ================= Phase B: FFN1 + GELU =================
            with tc.tile_pool(name="w1", bufs=1) as w1pool, \
                 tc.tile_pool(name="bh", bufs=2) as bh, \
                 tc.tile_pool(name="by", bufs=1) as by, \
                 tc.tile_pool(name="psb", bufs=6, space="PSUM") as ps_b:

                w1_sb = w1pool.tile([128, 8, FF], f32r, tag="w1")
                for cc in range(8):
                    nc.sync.dma_start(w1_sb[:, cc, :], w1_d[:, cc, :].bitcast(f32r))

                for b in range(BB):
                    h2l = bh.tile([128, 8, T], f32r, tag="h2l")
                    nc.sync.dma_start(h2l[:, :, :], h2T_d[b, :, :, :])
                    y1 = by.tile([128, 32, T], f32r, tag="y1")
                    for nch in range(32):
                        p1 = ps_b.tile([128, 256], f32, tag="b")
                        for cc in range(8):
                            nc.tensor.matmul(p1[:, :],
                                             w1_sb[:, cc, nch * 128:(nch + 1) * 128],
                                             h2l[:, cc, :],
                                             start=cc == 0, stop=cc == 7)
                        nc.scalar.activation(out=y1[:, nch, :], in_=p1[:, :],
                                             func=AF.Gelu,
                                             bias=b1_sb[:, nch:nch + 1])
                    nc.sync.dma_start(y1g_d[b, :, :, :], y1[:, :, :])

            # ================= Phase C: FFN2 + residual =================
            with tc.tile_pool(name="w2", bufs=1) as w2pool, \
                 tc.tile_pool(name="cy", bufs=1) as cy, \
                 tc.tile_pool(name="cx2", bufs=2) as cx2, \
                 tc.tile_pool(name="cout", bufs=2) as cout, \
                 tc.tile_pool(name="psc", bufs=6, space="PSUM") as ps_c:

                w2_sb = w2pool.tile([128, 32, C], f32r, tag="w2")
                for m in range(0, 32, 4):
                    nc.sync.dma_start(w2_sb[:, m:m + 4, :],
                                      w2_d[:, m:m + 4, :].bitcast(f32r))

                for b in range(BB):
                    yl = cy.tile([128, 32, T], f32r, tag="yl")
                    nc.sync.dma_start(yl[:, :, :], y1g_d[b, :, :, :])
                    x2l = cx2.tile([128, 2, C], f32, tag="x2l")
                    nc.sync.dma_start(x2l[:, :, :], x2_d[b, :, :, :])
                    for tcb in range(2):
                        nc.vector.tensor_add(x2l[:, tcb, :], x2l[:, tcb, :],
                                             b2_sb[:, :])
                    ot = cout.tile([128, 2, C], f32, tag="ot")
                    for tcb in range(2):
                        for ch in range(2):
                            p2 = ps_c.tile([128, 512], f32, tag="c")
                            for m in range(32):
                                nc.tensor.matmul(
                                    p2[:, :],
                                    yl[:, m, tcb * 128:(tcb + 1) * 128],
                                    w2_sb[:, m, ch * 512:(ch + 1) * 512],
                                    start=m == 0, stop=m == 31)
                            nc.vector.tensor_add(ot[:, tcb, ch * 512:(ch + 1) * 512],
                                                 p2[:, :],
                                                 x2l[:, tcb, ch * 512:(ch + 1) * 512])
                    nc.sync.dma_start(out_d[b, :, :, :], ot[:, :, :])

    nc.finalize()
    return nc


def _prep_inputs(x, ln1_g, ln1_b, wq, wk, wv, w_proj, b_proj, ln2_g, ln2_b,
                 w1, b1, w2, b2):
    """Host-side re-layouts shared by all cores."""
    common = {
        # [h, c, d] -> [p, cc, (h d)]
        "wq_r": np.ascontiguousarray(
            wq.reshape(H, 8, 128, HD).transpose(2, 1, 0, 3).reshape(128, 8, C)),
        "wk_r": np.ascontiguousarray(
            wk.reshape(H, 8, 128, HD).transpose(2, 1, 0, 3).reshape(128, 8, C)),
        "wv_r": np.ascontiguousarray(
            wv.reshape(H, 8, 128, HD).transpose(2, 1, 0, 3).reshape(128, 8, C)),
        "wp_r": np.ascontiguousarray(w_proj.reshape(8, 128, C).transpose(1, 0, 2)),
        "w1_r": np.ascontiguousarray(w1.reshape(8, 128, FF).transpose(1, 0, 2)),
        "w2_r": np.ascontiguousarray(w2.reshape(32, 128, C).transpose(1, 0, 2)),
        "ln1g_r": np.ascontiguousarray(ln1_g.reshape(8, 128).T),
        "ln1b_r": np.ascontiguousarray(ln1_b.reshape(8, 128).T),
        "ln2g_r": np.ascontiguousarray(ln2_g.reshape(8, 128).T),
        "ln2b_r": np.ascontiguousarray(ln2_b.reshape(8, 128).T),
        "b1_r": np.ascontiguousarray(b1.reshape(32, 128).T),
        "bproj_bc": np.ascontiguousarray(
            np.broadcast_to(b_proj, (128, C)).astype(np.float32)),
        "b2_bc": np.ascontiguousarray(
            np.broadcast_to(b2, (128, C)).astype(np.float32)),
        "cmask": np.triu(np.ones((128, 256), np.float32)),
        "cmask1": np.concatenate(
            [np.zeros((128, 128), np.float32),
             np.triu(np.ones((128, 128), np.float32))], axis=1),
        "ident": np.eye(128, dtype=np.float32),
        "ones128": np.ones((128, 1), np.float32),
        "ones64": np.ones((1, 64), np.float32),
    }
    in_maps = []
    for core in range(NCORES):
        xs = x[core * BB:(core + 1) * BB]  # [BB, 256, 1024]
        xr = np.ascontiguousarray(
            xs.reshape(BB, 2, 128, C).transpose(0, 2, 1, 3))
        in_maps.append({**common, "x_r": xr})
    return in_maps


def _run(inputs, trace=False):
    if "nc" not in _cache:
        _cache["nc"] = _build()
    nc = _cache["nc"]
    inputs = {k: np.asarray(v, dtype=np.float32) for k, v in inputs.items()}
    in_maps = _prep_inputs(**inputs)
    res = run_bass_kernel_spmd(nc, in_maps, core_ids=list(range(NCORES)),
                               trace=trace)
    outs = []
    for core in range(NCORES):
        o = res.results[core]["out_r"]  # [BB, 128, 2, C]
        outs.append(o.transpose(0, 2, 1, 3).reshape(BB, T, C))
    full = np.concatenate(outs, axis=0).astype(np.float32)
    return full, res


def kernel(**inputs):
    out, _ = _run(inputs, trace=False)
    return out


# revision 17
# speedup vs baseline: 1.3774x; 1.3774x over previous
"""Trainium2 Bass kernel for a dense transformer block.

B=64, T=256, C=1024, H=16, HD=64. Data-parallel over batch across 8 cores
(8 batch elements per core, processed as 4 batch-pairs so weight-stationary
matmuls get N=512 moving operands). All heavy matmuls run in fp32r (full PE
rate); attention is computed in scores-transposed layout so the softmax
reduction is a PE ones-matmul (replicated across 64 partitions so the
reciprocal runs wide) and no attention-matrix transpose is needed.
"""

import sys

if "/opt/trn_rl_repo" not in sys.path:
    sys.path.insert(0, "/opt/trn_rl_repo")

import numpy as np

import concourse.bass as bass
import concourse.tile as tile
from concourse import bacc, mybir
from concourse.bass_utils import run_bass_kernel_spmd

f32 = mybir.dt.float32
f32r = mybir.dt.float32r
AF = mybir.ActivationFunctionType
AL = mybir.AluOpType

NCORES = 8
BB = 8          # batch elements per core
NBP = BB // 2   # batch pairs per core
T = 256
TT = 512        # two batches' t-dim
C = 1024
H = 16
HD = 64
FF = 4096
EPS = 1e-5

_cache = {}


def _layernorm_transpose(nc, x_tile, g_sb, b_sb, hT_tile, ident_sb, small, ps_tr,
                         eps_sb):
    """x_tile [128,4,1024] f32 natural (rows on partitions, modified in place)
    -> hT_tile [128,8,512] f32r transposed (c on partitions), affine applied."""
    for tcb in range(4):
        stats = small.tile([128, 2, 6], f32, tag="stats")
        nc.vector.bn_stats(stats[:, 0, :], x_tile[:, tcb, 0:512])
        nc.vector.bn_stats(stats[:, 1, :], x_tile[:, tcb, 512:1024])
        mv = small.tile([128, 2], f32, tag="mv")
        nc.vector.bn_aggr(mv[:, :], stats[:, :, :])
        sd = small.tile([128, 1], f32, tag="sd")
        nc.scalar.activation(sd[:, :], mv[:, 1:2], AF.Sqrt, bias=eps_sb[:, :])
        rstd = small.tile([128, 1], f32, tag="rstd")
        nc.vector.reciprocal(rstd[:, :], sd[:, :])
        nc.vector.tensor_scalar(
            out=x_tile[:, tcb, :], in0=x_tile[:, tcb, :],
            scalar1=mv[:, 0:1], scalar2=rstd[:, :],
            op0=AL.subtract, op1=AL.mult,
        )
    for tcb in range(4):
        for cc in range(8):
            pt = ps_tr.tile([128, 128], f32, tag="tr")
            nc.tensor.transpose(pt[:, :], x_tile[:, tcb, cc * 128:(cc + 1) * 128],
                                ident_sb[:, :])
            nc.scalar.activation(
                out=hT_tile[:, cc, tcb * 128:(tcb + 1) * 128], in_=pt[:, :],
                func=AF.Identity, scale=g_sb[:, cc:cc + 1], bias=b_sb[:, cc:cc + 1],
            )


def _build():
    nc = bacc.Bacc("TRN2", target_bir_lowering=False, debug=False,
                   num_devices=NCORES)

    # ---- external IO (host pre-rearranged layouts) ----
    x_d = nc.dram_tensor("x_r", [NBP, 128, 4, C], f32, kind="ExternalInput")
    wq_d = nc.dram_tensor("wq_r", [128, 8, C], f32, kind="ExternalInput")
    wk_d = nc.dram_tensor("wk_r", [128, 8, C], f32, kind="ExternalInput")
    wv_d = nc.dram_tensor("wv_r", [128, 8, C], f32, kind="ExternalInput")
    wp_d = nc.dram_tensor("wp_r", [128, 8, C], f32, kind="ExternalInput")
    w1_d = nc.dram_tensor("w1_r", [128, 8, FF], f32, kind="ExternalInput")
    w2_d = nc.dram_tensor("w2_r", [128, 32, C], f32, kind="ExternalInput")
    ln1g_d = nc.dram_tensor("ln1g_r", [128, 8], f32, kind="ExternalInput")
    ln1b_d = nc.dram_tensor("ln1b_r", [128, 8], f32, kind="ExternalInput")
    ln2g_d = nc.dram_tensor("ln2g_r", [128, 8], f32, kind="ExternalInput")
    ln2b_d = nc.dram_tensor("ln2b_r", [128, 8], f32, kind="ExternalInput")
    b1_d = nc.dram_tensor("b1_r", [128, 32], f32, kind="ExternalInput")
    bproj_d = nc.dram_tensor("bproj_bc", [128, C], f32, kind="ExternalInput")
    b2_d = nc.dram_tensor("b2_bc", [128, C], f32, kind="ExternalInput")
    cmask_d = nc.dram_tensor("cmask01", [128, TT], f32, kind="ExternalInput")
    ident_d = nc.dram_tensor("ident", [128, 128], f32, kind="ExternalInput")
    onesrep_d = nc.dram_tensor("ones_rep", [128, 64], f32, kind="ExternalInput")
    out_d = nc.dram_tensor("out_r", [NBP, 128, 4, C], f32, kind="ExternalOutput")

    # ---- internal DRAM scratch (per-phase spills) ----
    oT_d = nc.dram_tensor("oT_s", [NBP, 128, 8, TT], f32r, kind="Internal")
    x2_d = nc.dram_tensor("x2_s", [NBP, 128, 4, C], f32, kind="Internal")
    h2T_d = nc.dram_tensor("h2T_s", [NBP, 128, 8, TT], f32r, kind="Internal")
    y1g_d = nc.dram_tensor("y1g_s", [NBP, 128, 32, TT], f32r, kind="Internal")

    with tile.TileContext(nc) as tc:
        with tc.tile_pool(name="const", bufs=1) as cpool:
            ident_sb = cpool.tile([128, 128], f32, tag="ident")
            nc.sync.dma_start(ident_sb[:, :], ident_d[:, :])
            cmask_sb = cpool.tile([128, TT], f32r, tag="cmask")
            nc.sync.dma_start(cmask_sb[:, :], cmask_d[:, :].bitcast(f32r))
            onesrep_sb = cpool.tile([128, 64], f32r, tag="onesrep")
            nc.sync.dma_start(onesrep_sb[:, :], onesrep_d[:, :].bitcast(f32r))
            ln1g_sb = cpool.tile([128, 8], f32, tag="ln1g")
            nc.sync.dma_start(ln1g_sb[:, :], ln1g_d[:, :])
            ln1b_sb = cpool.tile([128, 8], f32, tag="ln1b")
            nc.sync.dma_start(ln1b_sb[:, :], ln1b_d[:, :])
            ln2g_sb = cpool.tile([128, 8], f32, tag="ln2g")
            nc.sync.dma_start(ln2g_sb[:, :], ln2g_d[:, :])
            ln2b_sb = cpool.tile([128, 8], f32, tag="ln2b")
            nc.sync.dma_start(ln2b_sb[:, :], ln2b_d[:, :])
            eps_sb = cpool.tile([128, 1], f32, tag="eps")
            nc.vector.memset(eps_sb[:, :], EPS)

            # ================= Phase A1: LN1 + QKV + attention =================
            with tc.tile_pool(name="wqkv", bufs=1) as wpool, \
                 tc.tile_pool(name="a1x", bufs=1) as a1x, \
                 tc.tile_pool(name="a1h", bufs=1) as a1h, \
                 tc.tile_pool(name="a1qkv", bufs=1) as a1qkv, \
                 tc.tile_pool(name="a1e", bufs=5) as a1e, \
                 tc.tile_pool(name="a1rb", bufs=4) as a1rb, \
                 tc.tile_pool(name="a1o", bufs=1) as a1o, \
                 tc.tile_pool(name="a1small", bufs=4) as a1small, \
                 tc.tile_pool(name="psa_tr", bufs=1, space="PSUM") as ps_tr, \
                 tc.tile_pool(name="psa_mm", bufs=2, space="PSUM") as ps_mm, \
                 tc.tile_pool(name="psa_po", bufs=2, space="PSUM") as ps_po, \
                 tc.tile_pool(name="psa_pd", bufs=2, space="PSUM") as ps_pd, \
                 tc.tile_pool(name="psa_v", bufs=1, space="PSUM") as ps_v:

                wq_sb = wpool.tile([128, 8, C], f32r, tag="wq")
                wk_sb = wpool.tile([128, 8, C], f32r, tag="wk")
                wv_sb = wpool.tile([128, 8, C], f32r, tag="wv")
                for cc in range(8):
                    nc.sync.dma_start(wq_sb[:, cc, :], wq_d[:, cc, :].bitcast(f32r))
                    nc.sync.dma_start(wk_sb[:, cc, :], wk_d[:, cc, :].bitcast(f32r))
                    nc.sync.dma_start(wv_sb[:, cc, :], wv_d[:, cc, :].bitcast(f32r))

                for bp in range(NBP):
                    xn = a1x.tile([128, 4, C], f32, tag="xn")
                    nc.sync.dma_start(xn[:, :, :], x_d[bp, :, :, :])
                    hT = a1h.tile([128, 8, TT], f32r, tag="hT")
                    _layernorm_transpose(nc, xn, ln1g_sb, ln1b_sb, hT,
                                         ident_sb, a1small, ps_tr, eps_sb)

                    qT = a1qkv.tile([128, 8, TT], f32r, tag="qT")
                    kT = a1qkv.tile([128, 8, TT], f32r, tag="kT")
                    v_sb = a1qkv.tile([128, 4, C], f32r, tag="v")
                    for hp in range(8):
                        pq = ps_mm.tile([128, TT], f32, tag="mm")
                        for cc in range(8):
                            nc.tensor.matmul(pq[:, :],
                                             wq_sb[:, cc, hp * 128:(hp + 1) * 128],
                                             hT[:, cc, :],
                                             start=cc == 0, stop=cc == 7)
                        nc.vector.tensor_copy(qT[:, hp, :], pq[:, :])
                        pk = ps_mm.tile([128, TT], f32, tag="mm")
                        for cc in range(8):
                            nc.tensor.matmul(pk[:, :],
                                             wk_sb[:, cc, hp * 128:(hp + 1) * 128],
                                             hT[:, cc, :],
                                             start=cc == 0, stop=cc == 7)
                        nc.vector.tensor_copy(kT[:, hp, :], pk[:, :])
                    for sc4 in range(4):
                        for vh in range(2):
                            pv = ps_v.tile([128, 512], f32, tag="v")
                            for cc in range(8):
                                nc.tensor.matmul(pv[:, :],
                                                 hT[:, cc, sc4 * 128:(sc4 + 1) * 128],
                                                 wv_sb[:, cc, vh * 512:(vh + 1) * 512],
                                                 start=cc == 0, stop=cc == 7)
                            nc.vector.tensor_copy(v_sb[:, sc4, vh * 512:(vh + 1) * 512],
                                                  pv[:, :])

                    for bloc in range(2):
                        tsl = slice(bloc * 256, (bloc + 1) * 256)
                        oT = a1o.tile([128, 8, 256], f32r, tag="oT")
                        for hp in range(8):
                            for hl in range(2):
                                h = 2 * hp + hl
                                dsl = slice(hl * 64, (hl + 1) * 64)
                                # scoresT for both s-chunks into one [128,512]
                                ps_s = ps_mm.tile([128, TT], f32, tag="mm")
                                for sc in range(2):
                                    nc.tensor.matmul(
                                        ps_s[:, sc * 256:(sc + 1) * 256],
                                        kT[dsl, hp,
                                           bloc * 256 + sc * 128:
                                           bloc * 256 + (sc + 1) * 128],
                                        qT[dsl, hp, tsl],
                                        start=True, stop=True)
                                eh = a1e.tile([128, TT], f32r, tag="exp")
                                nc.scalar.activation(
                                    out=eh[:, :], in_=ps_s[:, :],
                                    func=AF.Exp, scale=0.125)
                                nc.gpsimd.tensor_mul(eh[:, :], eh[:, :],
                                                     cmask_sb[:, :])
                                # o_raw^T for this head: [64, 256] psum
                                poh = ps_po.tile([64, 256], f32, tag="po")
                                # replicated denominator: [64, 256] psum
                                pdh = ps_pd.tile([64, 256], f32, tag="pd")
                                for sc in range(2):
                                    sidx = bloc * 2 + sc
                                    nc.tensor.matmul(
                                        poh[:, :],
                                        v_sb[:, sidx, h * 64:(h + 1) * 64],
                                        eh[:, sc * 256:(sc + 1) * 256],
                                        start=sc == 0, stop=sc == 1)
                                    nc.tensor.matmul(
                                        pdh[:, :],
                                        onesrep_sb[:, :],
                                        eh[:, sc * 256:(sc + 1) * 256],
                                        start=sc == 0, stop=sc == 1)
                                rbh = a1rb.tile([64, 256], f32, tag="rb")
                                with nc.allow_low_precision(
                                        reason="softmax denom; 18 bits is ample"):
                                    nc.vector.reciprocal_approx_fast(
                                        rbh[:, :], pdh[:, :])
                                if hl == 0:
                                    nc.vector.tensor_mul(oT[0:64, hp, :],
                                                         poh[:, :], rbh[:, :])
                                else:
                                    tmpb = a1rb.tile([64, 256], f32r, tag="tmpb")
                                    nc.vector.tensor_mul(tmpb[:, :],
                                                         poh[:, :], rbh[:, :])
                                    nc.sync.dma_start(oT[64:128, hp, :],
                                                      tmpb[:, :])
                        nc.sync.dma_start(oT_d[bp, :, :, tsl], oT[:, :, :])

            # ================= Phase A2: proj + residual + LN2 =================
            with tc.tile_pool(name="wp", bufs=1) as wppool, \
                 tc.tile_pool(name="a2o", bufs=2) as a2o, \
                 tc.tile_pool(name="a2x", bufs=2) as a2x, \
                 tc.tile_pool(name="a2x2", bufs=2) as a2x2, \
                 tc.tile_pool(name="a2h", bufs=2) as a2h, \
                 tc.tile_pool(name="a2small", bufs=4) as a2small, \
                 tc.tile_pool(name="psa2_tr", bufs=2, space="PSUM") as ps2_tr, \
                 tc.tile_pool(name="psa2", bufs=4, space="PSUM") as ps_a2:

                bproj_sb = wppool.tile([128, C], f32, tag="bproj")
                nc.sync.dma_start(bproj_sb[:, :], bproj_d[:, :])
                wp_sb = wppool.tile([128, 8, C], f32r, tag="wp")
                for cc in range(8):
                    nc.sync.dma_start(wp_sb[:, cc, :], wp_d[:, cc, :].bitcast(f32r))

                for bp in range(NBP):
                    oT_l = a2o.tile([128, 8, TT], f32r, tag="oTl")
                    nc.sync.dma_start(oT_l[:, :, :], oT_d[bp, :, :, :])
                    xn = a2x.tile([128, 4, C], f32, tag="xn2")
                    nc.sync.dma_start(xn[:, :, :], x_d[bp, :, :, :])
                    for tcb in range(4):
                        nc.vector.tensor_add(xn[:, tcb, :], xn[:, tcb, :],
                                             bproj_sb[:, :])
                    x2 = a2x2.tile([128, 4, C], f32, tag="x2")
                    for tcb in range(4):
                        for ch in range(2):
                            pp = ps_a2.tile([128, 512], f32, tag="pj")
                            for cc in range(8):
                                nc.tensor.matmul(
                                    pp[:, :],
                                    oT_l[:, cc, tcb * 128:(tcb + 1) * 128],
                                    wp_sb[:, cc, ch * 512:(ch + 1) * 512],
                                    start=cc == 0, stop=cc == 7)
                            nc.vector.tensor_add(x2[:, tcb, ch * 512:(ch + 1) * 512],
                                                 pp[:, :],
                                                 xn[:, tcb, ch * 512:(ch + 1) * 512])
                    nc.sync.dma_start(x2_d[bp, :, :, :], x2[:, :, :])
                    h2T = a2h.tile([128, 8, TT], f32r, tag="h2T")
                    _layernorm_transpose(nc, x2, ln2g_sb, ln2b_sb, h2T,
                                         ident_sb, a2small, ps2_tr, eps_sb)
                    nc.sync.dma_start(h2T_d[bp, :, :, :], h2T[:, :, :])

            # ================= Phase B: FFN1 + GELU =================
            with tc.tile_pool(name="w1", bufs=1) as w1pool, \
                 tc.tile_pool(name="bh", bufs=2) as bh, \
                 tc.tile_pool(name="by", bufs=2) as by, \
                 tc.tile_pool(name="psb", bufs=6, space="PSUM") as ps_b:

                b1_sb = w1pool.tile([128, 32], f32, tag="b1")
                nc.sync.dma_start(b1_sb[:, :], b1_d[:, :])
                w1_sb = w1pool.tile([128, 8, FF], f32r, tag="w1")
                for cc in range(8):
                    nc.sync.dma_start(w1_sb[:, cc, :], w1_d[:, cc, :].bitcast(f32r))

                for bp in range(NBP):
                    h2l = bh.tile([128, 8, TT], f32r, tag="h2l")
                    nc.sync.dma_start(h2l[:, :, :], h2T_d[bp, :, :, :])
                    for nq in range(4):  # quarters of the 32 n-chunks
                        y1 = by.tile([128, 8, TT], f32r, tag="y1")
                        for nl in range(8):
                            nch = nq * 8 + nl
                            p1 = ps_b.tile([128, TT], f32, tag="b")
                            for cc in range(8):
                                nc.tensor.matmul(
                                    p1[:, :],
                                    w1_sb[:, cc, nch * 128:(nch + 1) * 128],
                                    h2l[:, cc, :],
                                    start=cc == 0, stop=cc == 7)
                            nc.scalar.activation(out=y1[:, nl, :], in_=p1[:, :],
                                                 func=AF.Gelu,
                                                 bias=b1_sb[:, nch:nch + 1])
                        nc.sync.dma_start(y1g_d[bp, :, nq * 8:(nq + 1) * 8, :],
                                          y1[:, :, :])

            # ================= Phase C: FFN2 + residual =================
            with tc.tile_pool(name="w2", bufs=1) as w2pool, \
                 tc.tile_pool(name="cy", bufs=2) as cy, \
                 tc.tile_pool(name="cx2", bufs=1) as cx2, \
                 tc.tile_pool(name="cout", bufs=1) as cout, \
                 tc.tile_pool(name="psc", bufs=8, space="PSUM") as ps_c:

                b2_sb = w2pool.tile([128, C], f32, tag="b2")
                nc.sync.dma_start(b2_sb[:, :], b2_d[:, :])
                w2_sb = w2pool.tile([128, 32, C], f32r, tag="w2")
                for m in range(0, 32, 4):
                    nc.sync.dma_start(w2_sb[:, m:m + 4, :],
                                      w2_d[:, m:m + 4, :].bitcast(f32r))

                for bp in range(NBP):
                    x2l = cx2.tile([128, 4, C], f32, tag="x2l")
                    nc.sync.dma_start(x2l[:, :, :], x2_d[bp, :, :, :])
                    for tcb in range(4):
                        nc.vector.tensor_add(x2l[:, tcb, :], x2l[:, tcb, :],
                                             b2_sb[:, :])
                    # all 8 output groups live across the 4 y1g quarter loads
                    p2s = [ps_c.tile([128, 512], f32, tag="c", name=f"p2_{bp}_{g}")
                           for g in range(8)]
                    for mq in range(4):
                        yq = cy.tile([128, 8, TT], f32r, tag="yq")
                        nc.sync.dma_start(yq[:, :, :],
                                          y1g_d[bp, :, mq * 8:(mq + 1) * 8, :])
                        for tcb in range(4):
                            for ch in range(2):
                                p2 = p2s[tcb * 2 + ch]
                                for ml in range(8):
                                    m = mq * 8 + ml
                                    nc.tensor.matmul(
                                        p2[:, :],
                                        yq[:, ml, tcb * 128:(tcb + 1) * 128],
                                        w2_sb[:, m, ch * 512:(ch + 1) * 512],
                                        start=m == 0, stop=m == 31)
                    ot = cout.tile([128, 4, C], f32, tag="ot")
                    for tcb in range(4):
                        for ch in range(2):
                            nc.vector.tensor_add(ot[:, tcb, ch * 512:(ch + 1) * 512],
                                                 p2s[tcb * 2 + ch][:, :],
                                                 x2l[:, tcb, ch * 512:(ch + 1) * 512])
                    nc.sync.dma_start(out_d[bp, :, :, :], ot[:, :, :])

    nc.finalize()
    return nc


def _prep_inputs(x, ln1_g, ln1_b, wq, wk, wv, w_proj, b_proj, ln2_g, ln2_b,
                 w1, b1, w2, b2):
    """Host-side re-layouts shared by all cores."""
    cm0 = np.triu(np.ones((128, 256), np.float32))
    cm1 = np.concatenate([np.zeros((128, 128), np.float32),
                          np.triu(np.ones((128, 128), np.float32))], axis=1)
    common = {
        # [h, c, d] -> [p, cc, (h d)]
        "wq_r": np.ascontiguousarray(
            wq.reshape(H, 8, 128, HD).transpose(2, 1, 0, 3).reshape(128, 8, C)),
        "wk_r": np.ascontiguousarray(
            wk.reshape(H, 8, 128, HD).transpose(2, 1, 0, 3).reshape(128, 8, C)),
        "wv_r": np.ascontiguousarray(
            wv.reshape(H, 8, 128, HD).transpose(2, 1, 0, 3).reshape(128, 8, C)),
        "wp_r": np.ascontiguousarray(w_proj.reshape(8, 128, C).transpose(1, 0, 2)),
        "w1_r": np.ascontiguousarray(w1.reshape(8, 128, FF).transpose(1, 0, 2)),
        "w2_r": np.ascontiguousarray(w2.reshape(32, 128, C).transpose(1, 0, 2)),
        "ln1g_r": np.ascontiguousarray(ln1_g.reshape(8, 128).T),
        "ln1b_r": np.ascontiguousarray(ln1_b.reshape(8, 128).T),
        "ln2g_r": np.ascontiguousarray(ln2_g.reshape(8, 128).T),
        "ln2b_r": np.ascontiguousarray(ln2_b.reshape(8, 128).T),
        "b1_r": np.ascontiguousarray(b1.reshape(32, 128).T),
        "bproj_bc": np.ascontiguousarray(
            np.broadcast_to(b_proj, (128, C)).astype(np.float32)),
        "b2_bc": np.ascontiguousarray(
            np.broadcast_to(b2, (128, C)).astype(np.float32)),
        "cmask01": np.concatenate([cm0, cm1], axis=1),
        "ident": np.eye(128, dtype=np.float32),
        "ones_rep": np.ones((128, 64), np.float32),
    }
    in_maps = []
    for core in range(NCORES):
        xs = x[core * BB:(core + 1) * BB]  # [BB, 256, 1024]
        # [bp, b_local, tc, p, c] -> [bp, p, (b_local tc), c]
        xr = np.ascontiguousarray(
            xs.reshape(NBP, 2, 2, 128, C).transpose(0, 3, 1, 2, 4)
            .reshape(NBP, 128, 4, C))
        in_maps.append({**common, "x_r": xr})
    return in_maps


def _run(inputs, trace=False):
    if "nc" not in _cache:
        _cache["nc"] = _build()
    nc = _cache["nc"]
    inputs = {k: np.asarray(v, dtype=np.float32) for k, v in inputs.items()}
    in_maps = _prep_inputs(**inputs)
    res = run_bass_kernel_spmd(nc, in_maps, core_ids=list(range(NCORES)),
                               trace=trace)
    outs = []
    for core in range(NCORES):
        o = res.results[core]["out_r"]  # [NBP, 128, 4, C]
        outs.append(o.reshape(NBP, 128, 2, 2, C).transpose(0, 2, 3, 1, 4)
                    .reshape(BB, T, C))
    full = np.concatenate(outs, axis=0).astype(np.float32)
    return full, res


def kernel(**inputs):
    out, _ = _run(inputs, trace=False)
    return out
